# revision 18
# baseline (speedup 1.0000x reference)
"""Trainium2 Bass kernel for nn_CrossDConv (dense_cnn).

Math (per batch sample b, see reference):
  z = rot_w @ x + rot_b (1x1 conv, 3 out ch), BN over (B,H,W) batch stats,
  angles = spatial mean of z_norm, angle = tanh(sum_i angles)*pi/4,
  s = cos(angle); the 3x3x3 FFT-domain weight tensor is phase-rotated by
  exp(-i*beta*G) with beta = 2*pi*s/3, inverse-FFT'd, mid-slice taken ->
  per-sample 3x3 2D kernels; then a batch-as-groups conv2d (pad 1).

Sharding: data-parallel over B across 8 NeuronCores, one sample per core.
Cross-core work: only the BN batch statistics (an AllReduce of 6 floats).

Device pipeline per core:
  A) stream x (16,512,512) into SBUF in (c, y%8)-partition layout; per
     8-row chunk compute z0 = rot_w@x via a block-diagonal matmul
     (K=128=(c,p), M=24=(i,p)), then Square+accum (ACT) and sum (DVE)
     to get per-sample sum(z0) and sum(z0^2).  rot_b cancels in the BN
     algebra so it is never needed on device.
  B) AllReduce (add) of [S1[i], S2[i]] over the 8 cores.
  C) tiny-op chain: var -> rsqrt -> angles -> tanh -> s=cos(angle);
     build the 27x9 complex iFFT/phase matrix M from sin/cos LUT calls,
     contract with the (27,256) transposed FFT weights (2 small matmuls
     per half) -> w2d (256,9); PE-transpose + 54 small DMAs scatter the
     banded conv lhsT matrices (3 of them, one per kernel column dx).
  D) conv: 86 row-strips of 6 output rows; for each, copy 8 input rows
     from resident x into a (c,yw)-partition strip tile, run 3
     accumulating f32r matmuls (K=128, M=96=(ys,o), N=512), DMA the
     PSUM tile straight to HBM.
"""

import sys

for _p in ("/opt/trn_rl_repo", "/root/.axon_site/_ro/trn_rl_repo"):
    if _p not in sys.path:
        sys.path.insert(0, _p)

import numpy as np

import concourse.bacc as bacc
import concourse.mybir as mybir
import concourse.tile as tile
from concourse.bass_utils import run_bass_kernel_spmd

F32 = mybir.dt.float32
F32R = mybir.dt.float32r
AF = mybir.ActivationFunctionType
ALU = mybir.AluOpType
AX = mybir.AxisListType

B, C, O, K, H, W = 8, 16, 16, 3, 512, 512
NCORES = 8
HWPIX = H * W                    # 262144
NPIX = B * HWPIX                 # 2097152
BN_EPS = 1e-5
NCHUNK = H // 8                  # 64 chunks of 8 rows
WPAD = W + 2                     # row layout: [0pad, x0..x511, 0pad]
SROWS = 6                        # output rows per conv strip
NSTRIP = (H + SROWS - 1) // SROWS  # 86 (last strip has 2 valid rows)
PI = float(np.pi)


def _consts():
    """Host-precomputed, input-independent constants (baked into the NEFF)."""
    g = np.array([0, 1, -1], np.int64)          # 3*fftfreq(3)
    j1, j2, j3 = np.meshgrid(np.arange(3), np.arange(3), np.arange(3),
                             indexing="ij")
    G = (g[j1] + g[j2] + g[j3]).reshape(27)     # in [-3, 3]

    sel_cos = np.zeros((4, 27), np.float32)
    sel_sin = np.zeros((4, 27), np.float32)
    for j in range(27):
        a = abs(G[j])
        sel_cos[a, j] = 1.0
        if G[j] != 0:
            sgn = float(np.sign(G[j]))
            # sin_t[2] holds sin(2b - pi) = -sin(2b): fold the flip in here
            sel_sin[a, j] = -sgn if a == 2 else sgn

    u = np.arange(3)[None, :, None]
    v = np.arange(3)[None, None, :]
    cang = (2.0 * np.pi / 3.0) * (j1.reshape(27, 1, 1) * 1
                                  + j2.reshape(27, 1, 1) * u
                                  + j3.reshape(27, 1, 1) * v)
    cang = cang.reshape(27, 9)
    cosC = (np.cos(cang) / 27.0).astype(np.float32)
    sinC = (np.sin(cang) / 27.0).astype(np.float32)

    kconst = ((2.0 * np.pi / 3.0) * np.arange(4)).reshape(4, 1).astype(np.float32)
    shift_s = np.array([0.0, 0.0, -np.pi, -2.0 * np.pi], np.float32).reshape(4, 1)
    sigma = np.array([1.0, -1.0, -1.0, 1.0], np.float32).reshape(4, 1)
    tau = np.array([np.pi / 2, np.pi / 2, np.pi / 2, -1.5 * np.pi],
                   np.float32).reshape(4, 1)

    foldI = np.zeros((24, 3), np.float32)       # (i,p) -> i
    for k in range(24):
        foldI[k, k // 8] = 1.0

    ident = np.eye(128, dtype=np.float32)
    return dict(sel_cos=sel_cos, sel_sin=sel_sin, cosC=cosC, sinC=sinC,
                kconst=kconst, shift_s=shift_s, sigma=sigma, tau=tau,
                foldI=foldI, ident=ident)


def build_nc(use_f32r_conv=True, use_f32r_z0=True):
    nc = bacc.Bacc("TRN2", target_bir_lowering=False, debug=False,
                   num_devices=NCORES)

    x_in = nc.dram_tensor("x", [C, H, W], F32, kind="ExternalInput")
    wfr_in = nc.dram_tensor("w_fft_real", [O, C, K, K, K], F32,
                            kind="ExternalInput")
    wfi_in = nc.dram_tensor("w_fft_imag", [O, C, K, K, K], F32,
                            kind="ExternalInput")
    rotw_in = nc.dram_tensor("rot_w", [3, C], F32, kind="ExternalInput")
    gam_in = nc.dram_tensor("bn_gamma", [3], F32, kind="ExternalInput")
    bet_in = nc.dram_tensor("bn_beta", [3], F32, kind="ExternalInput")
    out_t = nc.dram_tensor("out", [O, H, W], F32, kind="ExternalOutput")

    cc_in = nc.dram_tensor("cc_in", [1, 8], F32)    # internal bounce
    cc_out = nc.dram_tensor("cc_out", [1, 8], F32)

    cst = _consts()
    c_selcos = nc.inline_tensor(cst["sel_cos"], "c_selcos")
    c_selsin = nc.inline_tensor(cst["sel_sin"], "c_selsin")
    c_cosC = nc.inline_tensor(cst["cosC"], "c_cosC")
    c_sinC = nc.inline_tensor(cst["sinC"], "c_sinC")
    c_kconst = nc.inline_tensor(cst["kconst"], "c_kconst")
    c_shift = nc.inline_tensor(cst["shift_s"], "c_shift")
    c_sigma = nc.inline_tensor(cst["sigma"], "c_sigma")
    c_tau = nc.inline_tensor(cst["tau"], "c_tau")
    c_foldI = nc.inline_tensor(cst["foldI"], "c_foldI")
    c_ident = nc.inline_tensor(cst["ident"], "c_ident")

    mdt = F32R if use_f32r_conv else F32

    with tile.TileContext(nc) as tc:
        with tc.tile_pool(name="persist", bufs=1) as pp:
            # resident input, (c, y mod 8) on partitions, 64 padded row-chunks
            x_res = pp.tile([128, NCHUNK * WPAD], mdt)
            lhsT_z = pp.tile([128, 24], mdt)
            wtt_re = pp.tile([27, 256], F32)
            wtt_im = pp.tile([27, 256], F32)
            s1cols = pp.tile([24, NCHUNK], F32)
            s2cols = pp.tile([24, NCHUNK], F32)
            ss = pp.tile([24, 2], F32)
            loc_s = pp.tile([1, 8], F32)
            tot_s = pp.tile([1, 8], F32)
            gam_sb = pp.tile([1, 3], F32)
            bet_sb = pp.tile([1, 3], F32)
            ident_sb = pp.tile([128, 128], F32)
            selcos_sb = pp.tile([4, 27], F32)
            selsin_sb = pp.tile([4, 27], F32)
            cosC_sb = pp.tile([27, 9], F32)
            sinC_sb = pp.tile([27, 9], F32)
            kconst_sb = pp.tile([4, 1], F32)
            shift_sb = pp.tile([4, 1], F32)
            sigma_sb = pp.tile([4, 1], F32)
            tau_sb = pp.tile([4, 1], F32)
            w2d_sb = pp.tile([128, 18], F32)
            lhsT_c = [pp.tile([128, 96], mdt, tag=f"lhsTc{dx}",
                              name=f"lhsT_c{dx}")
                      for dx in range(3)]
            sc3 = pp.tile([1, 3], F32, tag="sc3a")   # phase-C temporaries
            sc3b = pp.tile([1, 3], F32, tag="sc3b")
            sc3c = pp.tile([1, 3], F32, tag="sc3c")
            sc1 = pp.tile([1, 1], F32, tag="sc1a")
            sc1b = pp.tile([1, 1], F32, tag="sc1b")
            kb = pp.tile([4, 1], F32, tag="kb")
            s4 = pp.tile([4, 1], F32, tag="s4")
            sin_t = pp.tile([4, 1], F32, tag="sint")
            cos_t = pp.tile([4, 1], F32, tag="cost")
            bg = pp.tile([27, 2], F32, tag="bg")
            m_re = pp.tile([27, 9], F32, tag="mre")
            m_imn = pp.tile([27, 9], F32, tag="mimn")
            mt1 = pp.tile([27, 9], F32, tag="mt1")
            mt2 = pp.tile([27, 9], F32, tag="mt2")
            eps_sb = pp.tile([1, 1], F32, tag="eps_sb")
            nqpi_sb = pp.tile([1, 1], F32, tag="nqpi_sb")
            hpi_sb = pp.tile([1, 1], F32, tag="hpi_sb")
            nc.vector.memset(eps_sb[:], BN_EPS)
            nc.vector.memset(nqpi_sb[:], -PI / 4.0)
            nc.vector.memset(hpi_sb[:], PI / 2.0)

            # ---- one-time setup ----
            nc.sync.dma_start(ident_sb[:], c_ident.ap())
            nc.sync.dma_start(selcos_sb[:], c_selcos.ap())
            nc.sync.dma_start(selsin_sb[:], c_selsin.ap())
            nc.sync.dma_start(cosC_sb[:], c_cosC.ap())
            nc.sync.dma_start(sinC_sb[:], c_sinC.ap())
            nc.sync.dma_start(kconst_sb[:], c_kconst.ap())
            nc.sync.dma_start(shift_sb[:], c_shift.ap())
            nc.sync.dma_start(sigma_sb[:], c_sigma.ap())
            nc.sync.dma_start(tau_sb[:], c_tau.ap())
            nc.sync.dma_start(gam_sb[:], gam_in.ap().unsqueeze(0))
            nc.sync.dma_start(bet_sb[:], bet_in.ap().unsqueeze(0))
            # transposed FFT weights: [27, (c,o)] so the w2d matmul's PSUM
            # partition layout is (c', o) (c-high folded into the free dim).
            # One DMA per channel keeps every AP within the 3-dim DMA limit.
            wtt_src_re = wfr_in.ap().rearrange("o c a b d -> c (a b d) o")
            wtt_src_im = wfi_in.ap().rearrange("o c a b d -> c (a b d) o")
            for c in range(C):
                nc.sync.dma_start(wtt_re[:, c * O:(c + 1) * O],
                                  wtt_src_re[c])
                nc.sync.dma_start(wtt_im[:, c * O:(c + 1) * O],
                                  wtt_src_im[c])
            # block-diagonal rot_w: lhsT_z[(c,p), (i*8+p)] = rot_w[i, c]
            nc.vector.memset(lhsT_z[:].bitcast(F32), 0.0)
            rot_co = rotw_in.ap().rearrange("i c -> c i")
            for p in range(8):
                nc.sync.dma_start(lhsT_z[p::8, p:24:8], rot_co.bitcast(mdt))
            # zero the left/right pad column of every row-chunk
            xv = x_res[:].bitcast(F32).rearrange("p (g w) -> p g w", w=WPAD)
            nc.vector.memset(xv[:, :, 0:1], 0.0)
            nc.vector.memset(xv[:, :, WPAD - 1:WPAD], 0.0)
            nc.vector.memset(loc_s[:], 0.0)

            # ---- phase A: load x + z0 statistics ----
            with (
                tc.tile_pool(name="pa_psum", bufs=4, space="PSUM") as pza,
                tc.tile_pool(name="pa_scr", bufs=2) as psc,
            ):
                for g in range(NCHUNK):
                    dst = x_res[:, g * WPAD + 1: g * WPAD + 1 + W]
                    src = x_in.ap()[:, 8 * g: 8 * g + 8, :]
                    nc.sync.dma_start(dst, src.bitcast(mdt))
                    z0 = pza.tile([24, W], F32, tag="z0")
                    nc.tensor.matmul(z0[:], lhsT_z[:],
                                     x_res[:, g * WPAD + 1: g * WPAD + 1 + W],
                                     start=True, stop=True)
                    scr = psc.tile([24, W], mybir.dt.bfloat16, tag="scr")
                    nc.scalar.activation(scr[:], z0[:], AF.Square,
                                         accum_out=s2cols[:, g:g + 1])
                    nc.vector.reduce_sum(s1cols[:, g:g + 1], z0[:], axis=AX.X)

            # ---- phase A2 + B: fold + AllReduce ----
            with tc.tile_pool(name="pb_psum", bufs=1, space="PSUM") as pzb:
                nc.vector.reduce_sum(ss[:, 0:1], s1cols[:], axis=AX.X)
                nc.vector.reduce_sum(ss[:, 1:2], s2cols[:], axis=AX.X)
                pf = pzb.tile([3, 2], F32, tag="pf")
                foldI_sb = pp.tile([24, 3], F32, tag="foldI")
                nc.sync.dma_start(foldI_sb[:], c_foldI.ap())
                nc.tensor.matmul(pf[:], foldI_sb[:], ss[:],
                                 start=True, stop=True)
                pf_sb = pp.tile([3, 2], F32, tag="pf_sb")
                nc.vector.tensor_copy(pf_sb[:], pf[:])
                # interleaved (S1[0],S2[0],S1[1],S2[1],S1[2],S2[2])
                nc.sync.dma_start(loc_s[:, 0:6], pf_sb[:])
                nc.sync.dma_start(cc_in.ap(), loc_s[:])
                nc.gpsimd.collective_compute(
                    "AllReduce", ALU.add,
                    replica_groups=[list(range(NCORES))],
                    ins=[cc_in.ap()], outs=[cc_out.ap()])
                nc.sync.dma_start(tot_s[:], cc_out.ap())

                # ---- phase C: scalars -> rotation -> w2d -> conv lhsT ----
                t1 = tot_s[:, 0:6:2]     # sum z0   (over batch)
                t2 = tot_s[:, 1:6:2]     # sum z0^2 (over batch)
                nc.vector.tensor_scalar_mul(sc3[:], t1, 1.0 / NPIX)   # m1
                nc.vector.tensor_scalar_mul(sc3b[:], t2, 1.0 / NPIX)  # e2
                nc.vector.tensor_tensor(sc3c[:], sc3[:], sc3[:], op=ALU.mult)
                nc.vector.tensor_tensor(sc3b[:], sc3b[:], sc3c[:],
                                        op=ALU.subtract)              # var
                nc.scalar.activation(sc3b[:], sc3b[:], AF.Sqrt,
                                     bias=eps_sb[:])
                nc.vector.reciprocal(sc3b[:], sc3b[:])                # rsqrt
                nc.vector.tensor_tensor(sc3b[:], sc3b[:], gam_sb[:],
                                        op=ALU.mult)                  # inv
                nc.vector.tensor_scalar_mul(sc3c[:], loc_s[:, 0:6:2],
                                            1.0 / HWPIX)              # s1h
                nc.vector.tensor_tensor(sc3c[:], sc3c[:], sc3[:],
                                        op=ALU.subtract)              # diff
                nc.vector.tensor_tensor(sc3c[:], sc3c[:], sc3b[:],
                                        op=ALU.mult)
                nc.vector.tensor_tensor(sc3c[:], sc3c[:], bet_sb[:],
                                        op=ALU.add)                   # angles
                nc.vector.reduce_sum(sc1[:], sc3c[:], axis=AX.X)      # a
                nc.scalar.activation(sc1b[:], sc1[:], AF.Tanh)
                # s = cos(tanh(a)*pi/4) = sin(pi/2 - (pi/4)*tanh(a))
                nc.scalar.activation(sc1[:], sc1b[:], AF.Sin,
                                     scale=nqpi_sb[:], bias=hpi_sb[:])
                nc.gpsimd.partition_broadcast(s4[:], sc1[:])
                nc.vector.tensor_tensor(kb[:], kconst_sb[:], s4[:],
                                        op=ALU.mult)                  # k*beta
                nc.scalar.activation(sin_t[:], kb[:], AF.Sin,
                                     bias=shift_sb[:])
                nc.scalar.activation(cos_t[:], kb[:], AF.Sin,
                                     scale=sigma_sb[:], bias=tau_sb[:])
                pg0 = pzb.tile([27, 1], F32, tag="pg0")
                pg1 = pzb.tile([27, 1], F32, tag="pg1")
                nc.tensor.matmul(pg0[:], selcos_sb[:], cos_t[:],
                                 start=True, stop=True)
                nc.tensor.matmul(pg1[:], selsin_sb[:], sin_t[:],
                                 start=True, stop=True)
                nc.vector.tensor_copy(bg[:, 0:1], pg0[:])
                nc.vector.tensor_copy(bg[:, 1:2], pg1[:])
                # M_re = cosC*cbG + sinC*sbG ; M_imn = cosC*sbG - sinC*cbG
                nc.vector.tensor_scalar(mt1[:], cosC_sb[:], bg[:, 0:1], None,
                                        op0=ALU.mult)
                nc.vector.tensor_scalar(mt2[:], sinC_sb[:], bg[:, 1:2], None,
                                        op0=ALU.mult)
                nc.vector.tensor_tensor(m_re[:], mt1[:], mt2[:], op=ALU.add)
                nc.vector.tensor_scalar(mt1[:], cosC_sb[:], bg[:, 1:2], None,
                                        op0=ALU.mult)
                nc.vector.tensor_scalar(mt2[:], sinC_sb[:], bg[:, 0:1], None,
                                        op0=ALU.mult)
                nc.vector.tensor_tensor(m_imn[:], mt1[:], mt2[:],
                                        op=ALU.subtract)
                # w2d halves: psum partitions (c', o), free uv
                for h in range(2):
                    pw = pzb.tile([128, 9], F32, tag=f"pw{h}")
                    nc.tensor.matmul(pw[:], wtt_re[:, h * 128:(h + 1) * 128],
                                     m_re[:], start=True, stop=False)
                    nc.tensor.matmul(pw[:], wtt_im[:, h * 128:(h + 1) * 128],
                                     m_imn[:], start=False, stop=True)
                    nc.vector.tensor_copy(w2d_sb[:, h * 9:(h + 1) * 9], pw[:])
                # transpose -> [18=(h,uv), 128=(c',o)]
                pwt = pzb.tile([18, 128], F32, tag="pwt")
                nc.tensor.transpose(pwt[:], w2d_sb[:], ident_sb[:])
                pwt_sb = pp.tile([18, 128], F32, tag="pwt_sb")
                nc.vector.tensor_copy(pwt_sb[:], pwt[:])
                for dx in range(3):
                    nc.vector.memset(lhsT_c[dx][:].bitcast(F32), 0.0)
                for dx in range(3):
                    for dy in range(3):
                        src = pwt_sb[dy * 3 + dx::9, :] \
                            .rearrange("h (cp o) -> h cp o", o=O)
                        for ys in range(SROWS):
                            yw = ys + dy
                            nc.sync.dma_start(
                                lhsT_c[dx][yw::8, ys * O:(ys + 1) * O],
                                src.bitcast(mdt))

            # ---- phase D: the batch-as-groups conv ----
            with (
                tc.tile_pool(name="pd_strip", bufs=3) as pst,
                tc.tile_pool(name="pd_out", bufs=3) as pso,
                tc.tile_pool(name="pd_psum", bufs=4, space="PSUM") as pcv,
            ):
                for i in range(NSTRIP):
                    y0 = SROWS * i - 1
                    rv = min(SROWS, H - SROWS * i)
                    strip = pst.tile([128, WPAD], mdt, tag="strip")
                    if not (0 <= y0 and y0 + 7 < H):
                        # edge strip: zero everything, then fill valid rows
                        # (compute-engine APs need unit partition step, so a
                        # per-row-slot memset is not expressible)
                        nc.vector.memset(strip[:].bitcast(F32), 0.0)
                    for yw in range(8):
                        y = y0 + yw
                        if 0 <= y < H:
                            nc.sync.dma_start(
                                strip[yw::8, :],
                                x_res[(y % 8)::8,
                                      (y // 8) * WPAD:(y // 8 + 1) * WPAD])
                    pc = pcv.tile([96, W], F32, tag="pc")
                    for dx in range(3):
                        nc.tensor.matmul(pc[:], lhsT_c[dx][:],
                                         strip[:, dx:dx + W],
                                         start=(dx == 0), stop=(dx == 2))
                    osb = pso.tile([96, W], F32, tag="osb")
                    # PSUM has no DMA route; alternate evac engines
                    if i % 2 == 0:
                        nc.scalar.activation(osb[0:rv * O, :], pc[0:rv * O, :],
                                             AF.Copy)
                    else:
                        nc.vector.tensor_copy(osb[0:rv * O, :], pc[0:rv * O, :])
                    dst = out_t.ap()[:, SROWS * i: SROWS * i + rv, :] \
                        .rearrange("o y w -> y o w")
                    nc.sync.dma_start(dst, osb[0:rv * O, :])

    nc.compile()
    return nc


_NC_CACHE = {}


def _get_nc(key=(True, True)):
    if key not in _NC_CACHE:
        _NC_CACHE[key] = build_nc(*key)
    return _NC_CACHE[key]


def _install_ntff_hook():
    """Shim the missing antenv.axon_hooks so trace=True can profile."""
    try:
        import antenv.axon_hooks  # noqa: F401
        return
    except ImportError:
        pass
    import types

    import antenv

    if "/root/.axon_site" not in sys.path:
        sys.path.insert(0, "/root/.axon_site")
    from trn_agent_boot.trn_boot import _ntff_profile_via_ctypes

    hook = _ntff_profile_via_ctypes("/opt/axon/libaxon_pjrt.so")
    m = types.ModuleType("antenv.axon_hooks")
    holder = {"h": hook}
    m.get_axon_ntff_profile_hook = lambda: holder["h"]
    m.set_axon_ntff_profile_hook = lambda h: holder.__setitem__("h", h)
    sys.modules["antenv.axon_hooks"] = m
    antenv.axon_hooks = m


def run_kernel(inputs, trace=False, trace_kwargs=None):
    nc = _get_nc()
    if trace:
        try:
            _install_ntff_hook()
        except Exception as e:
            print(f"ntff hook install failed ({e}); tracing may be skipped")
    x = np.asarray(inputs["x"], np.float32)
    shared = {
        "w_fft_real": np.ascontiguousarray(inputs["w_fft_real"], np.float32),
        "w_fft_imag": np.ascontiguousarray(inputs["w_fft_imag"], np.float32),
        "rot_w": np.ascontiguousarray(inputs["rot_w"], np.float32),
        "bn_gamma": np.ascontiguousarray(inputs["bn_gamma"], np.float32),
        "bn_beta": np.ascontiguousarray(inputs["bn_beta"], np.float32),
    }
    in_maps = [dict(x=np.ascontiguousarray(x[b]), **shared) for b in range(B)]
    kw = {}
    if trace:
        kw = dict(trace=True, **(trace_kwargs or {}))
    res = run_bass_kernel_spmd(nc, in_maps, list(range(NCORES)), **kw)
    out = np.stack([res.results[b]["out"] for b in range(B)], axis=0)
    return out, res


def kernel(**inputs):
    out, _ = run_kernel(inputs)
    return out


# revision 20
# speedup vs baseline: 1.8223x; 1.8223x over previous
"""Trainium2 Bass kernel for nn_CrossDConv (dense_cnn).

Math (per batch sample b, see reference):
  z = rot_w @ x + rot_b (1x1 conv, 3 out ch), BN over (B,H,W) batch stats,
  angles = spatial mean of z_norm, angle = tanh(sum_i angles)*pi/4,
  s = cos(angle); the 3x3x3 FFT-domain weight tensor is phase-rotated by
  exp(-i*beta*G) with beta = 2*pi*s/3, inverse-FFT'd, mid-slice taken ->
  per-sample 3x3 2D kernels; then a batch-as-groups conv2d (pad 1).

Sharding: data-parallel over B across 8 NeuronCores, one sample per core.
Cross-core work: only the BN batch statistics (an AllReduce of 6 floats).

Device pipeline per core:
  A) stream x (16,512,512) into SBUF in (c, y%8)-partition layout; per
     8-row chunk compute z0 = rot_w@x via a block-diagonal matmul
     (K=128=(c,p), M=24=(i,p)), then Square+accum (ACT) and sum (DVE)
     to get per-sample sum(z0) and sum(z0^2).  rot_b cancels in the BN
     algebra so it is never needed on device.
  B) AllReduce (add) of [S1[i], S2[i]] over the 8 cores.
  C) tiny-op chain: var -> rsqrt -> angles -> tanh -> s=cos(angle);
     build the 27x9 complex iFFT/phase matrix M from sin/cos LUT calls,
     contract with the (27,256) transposed FFT weights (2 small matmuls
     per half) -> w2d (256,9); PE-transpose + 54 small DMAs scatter the
     banded conv lhsT matrices (3 of them, one per kernel column dx).
  D) conv: 86 row-strips of 6 output rows; for each, copy 8 input rows
     from resident x into a (c,yw)-partition strip tile, run 3
     accumulating f32r matmuls (K=128, M=96=(ys,o), N=512), DMA the
     PSUM tile straight to HBM.
"""

import sys

for _p in ("/opt/trn_rl_repo", "/root/.axon_site/_ro/trn_rl_repo"):
    if _p not in sys.path:
        sys.path.insert(0, _p)

import numpy as np

import concourse.bacc as bacc
import concourse.mybir as mybir
import concourse.tile as tile
from concourse.bass_utils import run_bass_kernel_spmd

F32 = mybir.dt.float32
F32R = mybir.dt.float32r
AF = mybir.ActivationFunctionType
ALU = mybir.AluOpType
AX = mybir.AxisListType

B, C, O, K, H, W = 8, 16, 16, 3, 512, 512
NCORES = 8
HWPIX = H * W                    # 262144
NPIX = B * HWPIX                 # 2097152
BN_EPS = 1e-5
NCHUNK = H // 8                  # 64 chunks of 8 rows
WPAD = W + 2                     # row layout: [0pad, x0..x511, 0pad]
SROWS = 6                        # output rows per conv strip
NSTRIP = (H + SROWS - 1) // SROWS  # 86 (last strip has 2 valid rows)
PI = float(np.pi)


def _consts():
    """Host-precomputed, input-independent constants (baked into the NEFF)."""
    g = np.array([0, 1, -1], np.int64)          # 3*fftfreq(3)
    j1, j2, j3 = np.meshgrid(np.arange(3), np.arange(3), np.arange(3),
                             indexing="ij")
    G = (g[j1] + g[j2] + g[j3]).reshape(27)     # in [-3, 3]

    sel_cos = np.zeros((4, 27), np.float32)
    sel_sin = np.zeros((4, 27), np.float32)
    for j in range(27):
        a = abs(G[j])
        sel_cos[a, j] = 1.0
        if G[j] != 0:
            sgn = float(np.sign(G[j]))
            # sin_t[2] holds sin(2b - pi) = -sin(2b): fold the flip in here
            sel_sin[a, j] = -sgn if a == 2 else sgn

    u = np.arange(3)[None, :, None]
    v = np.arange(3)[None, None, :]
    cang = (2.0 * np.pi / 3.0) * (j1.reshape(27, 1, 1) * 1
                                  + j2.reshape(27, 1, 1) * u
                                  + j3.reshape(27, 1, 1) * v)
    cang = cang.reshape(27, 9)
    cosC = (np.cos(cang) / 27.0).astype(np.float32)
    sinC = (np.sin(cang) / 27.0).astype(np.float32)

    kconst = ((2.0 * np.pi / 3.0) * np.arange(4)).reshape(4, 1).astype(np.float32)
    shift_s = np.array([0.0, 0.0, -np.pi, -2.0 * np.pi], np.float32).reshape(4, 1)
    sigma = np.array([1.0, -1.0, -1.0, 1.0], np.float32).reshape(4, 1)
    tau = np.array([np.pi / 2, np.pi / 2, np.pi / 2, -1.5 * np.pi],
                   np.float32).reshape(4, 1)

    foldI = np.zeros((24, 3), np.float32)       # (i,p) -> i
    for k in range(24):
        foldI[k, k // 8] = 1.0

    ident = np.eye(128, dtype=np.float32)
    return dict(sel_cos=sel_cos, sel_sin=sel_sin, cosC=cosC, sinC=sinC,
                kconst=kconst, shift_s=shift_s, sigma=sigma, tau=tau,
                foldI=foldI, ident=ident)


def build_nc(use_f32r_conv=True, use_f32r_z0=True):
    nc = bacc.Bacc("TRN2", target_bir_lowering=False, debug=False,
                   num_devices=NCORES)

    x_in = nc.dram_tensor("x", [C, H, W], F32, kind="ExternalInput")
    wfr_in = nc.dram_tensor("w_fft_real", [O, C, K, K, K], F32,
                            kind="ExternalInput")
    wfi_in = nc.dram_tensor("w_fft_imag", [O, C, K, K, K], F32,
                            kind="ExternalInput")
    rotw_in = nc.dram_tensor("rot_w", [3, C], F32, kind="ExternalInput")
    gam_in = nc.dram_tensor("bn_gamma", [3], F32, kind="ExternalInput")
    bet_in = nc.dram_tensor("bn_beta", [3], F32, kind="ExternalInput")
    out_t = nc.dram_tensor("out", [O, H, W], F32, kind="ExternalOutput")

    cc_in = nc.dram_tensor("cc_in", [1, 8], F32)    # internal bounce
    cc_out = nc.dram_tensor("cc_out", [1, 8], F32)

    cst = _consts()
    c_selcos = nc.inline_tensor(cst["sel_cos"], "c_selcos")
    c_selsin = nc.inline_tensor(cst["sel_sin"], "c_selsin")
    c_cosC = nc.inline_tensor(cst["cosC"], "c_cosC")
    c_sinC = nc.inline_tensor(cst["sinC"], "c_sinC")
    c_kconst = nc.inline_tensor(cst["kconst"], "c_kconst")
    c_shift = nc.inline_tensor(cst["shift_s"], "c_shift")
    c_sigma = nc.inline_tensor(cst["sigma"], "c_sigma")
    c_tau = nc.inline_tensor(cst["tau"], "c_tau")
    c_foldI = nc.inline_tensor(cst["foldI"], "c_foldI")
    c_ident = nc.inline_tensor(cst["ident"], "c_ident")

    mdt = F32R if use_f32r_conv else F32

    with tile.TileContext(nc) as tc:
        with tc.tile_pool(name="persist", bufs=1) as pp:
            lhsT_z = pp.tile([128, 24], mdt)
            wtt_re = pp.tile([27, 256], F32)
            wtt_im = pp.tile([27, 256], F32)
            s1cols = pp.tile([24, NCHUNK], F32)
            s2cols = pp.tile([24, NCHUNK], F32)
            ss = pp.tile([24, 2], F32)
            loc_s = pp.tile([1, 8], F32)
            tot_s = pp.tile([1, 8], F32)
            gam_sb = pp.tile([1, 3], F32)
            bet_sb = pp.tile([1, 3], F32)
            ident_sb = pp.tile([128, 128], F32)
            selcos_sb = pp.tile([4, 27], F32)
            selsin_sb = pp.tile([4, 27], F32)
            cosC_sb = pp.tile([27, 9], F32)
            sinC_sb = pp.tile([27, 9], F32)
            kconst_sb = pp.tile([4, 1], F32)
            shift_sb = pp.tile([4, 1], F32)
            sigma_sb = pp.tile([4, 1], F32)
            tau_sb = pp.tile([4, 1], F32)
            w2d_sb = pp.tile([128, 18], F32)
            lhsT_c = [pp.tile([128, 96], mdt, tag=f"lhsTc{dx}",
                              name=f"lhsT_c{dx}")
                      for dx in range(3)]
            sc3 = pp.tile([1, 3], F32, tag="sc3a")   # phase-C temporaries
            sc3b = pp.tile([1, 3], F32, tag="sc3b")
            sc3c = pp.tile([1, 3], F32, tag="sc3c")
            sc1 = pp.tile([1, 1], F32, tag="sc1a")
            sc1b = pp.tile([1, 1], F32, tag="sc1b")
            kb = pp.tile([4, 1], F32, tag="kb")
            s4 = pp.tile([4, 1], F32, tag="s4")
            sin_t = pp.tile([4, 1], F32, tag="sint")
            cos_t = pp.tile([4, 1], F32, tag="cost")
            bg = pp.tile([27, 2], F32, tag="bg")
            m_re = pp.tile([27, 9], F32, tag="mre")
            m_imn = pp.tile([27, 9], F32, tag="mimn")
            mt1 = pp.tile([27, 9], F32, tag="mt1")
            mt2 = pp.tile([27, 9], F32, tag="mt2")
            eps_sb = pp.tile([1, 1], F32, tag="eps_sb")
            nqpi_sb = pp.tile([1, 1], F32, tag="nqpi_sb")
            hpi_sb = pp.tile([1, 1], F32, tag="hpi_sb")
            nc.vector.memset(eps_sb[:], BN_EPS)
            nc.vector.memset(nqpi_sb[:], -PI / 4.0)
            nc.vector.memset(hpi_sb[:], PI / 2.0)

            # ---- one-time setup ----
            nc.sync.dma_start(ident_sb[:], c_ident.ap())
            nc.sync.dma_start(selcos_sb[:], c_selcos.ap())
            nc.sync.dma_start(selsin_sb[:], c_selsin.ap())
            nc.sync.dma_start(cosC_sb[:], c_cosC.ap())
            nc.sync.dma_start(sinC_sb[:], c_sinC.ap())
            nc.sync.dma_start(kconst_sb[:], c_kconst.ap())
            nc.sync.dma_start(shift_sb[:], c_shift.ap())
            nc.sync.dma_start(sigma_sb[:], c_sigma.ap())
            nc.sync.dma_start(tau_sb[:], c_tau.ap())
            nc.sync.dma_start(gam_sb[:], gam_in.ap().unsqueeze(0))
            nc.sync.dma_start(bet_sb[:], bet_in.ap().unsqueeze(0))
            # transposed FFT weights: [27, (c,o)] so the w2d matmul's PSUM
            # partition layout is (c', o) (c-high folded into the free dim).
            # One DMA per channel keeps every AP within the 3-dim DMA limit.
            wtt_src_re = wfr_in.ap().rearrange("o c a b d -> c (a b d) o")
            wtt_src_im = wfi_in.ap().rearrange("o c a b d -> c (a b d) o")
            for c in range(C):
                nc.sync.dma_start(wtt_re[:, c * O:(c + 1) * O],
                                  wtt_src_re[c])
                nc.sync.dma_start(wtt_im[:, c * O:(c + 1) * O],
                                  wtt_src_im[c])
            # block-diagonal rot_w: lhsT_z[(c,p), (i*8+p)] = rot_w[i, c]
            nc.vector.memset(lhsT_z[:].bitcast(F32), 0.0)
            rot_co = rotw_in.ap().rearrange("i c -> c i")
            for p in range(8):
                nc.sync.dma_start(lhsT_z[p::8, p:24:8], rot_co.bitcast(mdt))
            nc.vector.memset(loc_s[:], 0.0)
            # persistent conv strip buffers (manual 3-way rotation) with
            # zero pad columns written once
            strip_bufs = [pp.tile([128, WPAD], mdt, name=f"stripb{k}")
                          for k in range(3)]
            for sb in strip_bufs:
                nc.vector.memset(sb[:, 0:1].bitcast(F32), 0.0)
                nc.vector.memset(sb[:, WPAD - 1:WPAD].bitcast(F32), 0.0)

            # ---- phase A: load x + z0 statistics ----
            with (
                tc.tile_pool(name="pa_psum", bufs=4, space="PSUM") as pza,
                tc.tile_pool(name="pa_x", bufs=3) as pax,
                tc.tile_pool(name="pa_scr", bufs=2) as psc,
            ):
                for g in range(NCHUNK):
                    xch = pax.tile([128, W], mdt, tag="xchunk")
                    src = x_in.ap()[:, 8 * g: 8 * g + 8, :]
                    nc.sync.dma_start(xch[:], src.bitcast(mdt))
                    z0 = pza.tile([24, W], F32, tag="z0")
                    nc.tensor.matmul(z0[:], lhsT_z[:], xch[:],
                                     start=True, stop=True)
                    scr = psc.tile([24, W], mybir.dt.bfloat16, tag="scr")
                    nc.scalar.activation(scr[:], z0[:], AF.Square,
                                         accum_out=s2cols[:, g:g + 1])
                    nc.vector.reduce_sum(s1cols[:, g:g + 1], z0[:], axis=AX.X)

            # ---- phase A2 + B: fold + AllReduce ----
            with tc.tile_pool(name="pb_psum", bufs=1, space="PSUM") as pzb:
                nc.vector.reduce_sum(ss[:, 0:1], s1cols[:], axis=AX.X)
                nc.vector.reduce_sum(ss[:, 1:2], s2cols[:], axis=AX.X)
                pf = pzb.tile([3, 2], F32, tag="pf")
                foldI_sb = pp.tile([24, 3], F32, tag="foldI")
                nc.sync.dma_start(foldI_sb[:], c_foldI.ap())
                nc.tensor.matmul(pf[:], foldI_sb[:], ss[:],
                                 start=True, stop=True)
                pf_sb = pp.tile([3, 2], F32, tag="pf_sb")
                nc.vector.tensor_copy(pf_sb[:], pf[:])
                # interleaved (S1[0],S2[0],S1[1],S2[1],S1[2],S2[2])
                nc.sync.dma_start(loc_s[:, 0:6], pf_sb[:])
                nc.sync.dma_start(cc_in.ap(), loc_s[:])
                nc.gpsimd.collective_compute(
                    "AllReduce", ALU.add,
                    replica_groups=[list(range(NCORES))],
                    ins=[cc_in.ap()], outs=[cc_out.ap()])
                nc.sync.dma_start(tot_s[:], cc_out.ap())

                # ---- phase C: scalars -> rotation -> w2d -> conv lhsT ----
                t1 = tot_s[:, 0:6:2]     # sum z0   (over batch)
                t2 = tot_s[:, 1:6:2]     # sum z0^2 (over batch)
                nc.vector.tensor_scalar_mul(sc3[:], t1, 1.0 / NPIX)   # m1
                nc.vector.tensor_scalar_mul(sc3b[:], t2, 1.0 / NPIX)  # e2
                nc.vector.tensor_tensor(sc3c[:], sc3[:], sc3[:], op=ALU.mult)
                nc.vector.tensor_tensor(sc3b[:], sc3b[:], sc3c[:],
                                        op=ALU.subtract)              # var
                nc.scalar.activation(sc3b[:], sc3b[:], AF.Sqrt,
                                     bias=eps_sb[:])
                nc.vector.reciprocal(sc3b[:], sc3b[:])                # rsqrt
                nc.vector.tensor_tensor(sc3b[:], sc3b[:], gam_sb[:],
                                        op=ALU.mult)                  # inv
                nc.vector.tensor_scalar_mul(sc3c[:], loc_s[:, 0:6:2],
                                            1.0 / HWPIX)              # s1h
                nc.vector.tensor_tensor(sc3c[:], sc3c[:], sc3[:],
                                        op=ALU.subtract)              # diff
                nc.vector.tensor_tensor(sc3c[:], sc3c[:], sc3b[:],
                                        op=ALU.mult)
                nc.vector.tensor_tensor(sc3c[:], sc3c[:], bet_sb[:],
                                        op=ALU.add)                   # angles
                nc.vector.reduce_sum(sc1[:], sc3c[:], axis=AX.X)      # a
                nc.scalar.activation(sc1b[:], sc1[:], AF.Tanh)
                # s = cos(tanh(a)*pi/4) = sin(pi/2 - (pi/4)*tanh(a))
                nc.scalar.activation(sc1[:], sc1b[:], AF.Sin,
                                     scale=nqpi_sb[:], bias=hpi_sb[:])
                nc.gpsimd.partition_broadcast(s4[:], sc1[:])
                nc.vector.tensor_tensor(kb[:], kconst_sb[:], s4[:],
                                        op=ALU.mult)                  # k*beta
                nc.scalar.activation(sin_t[:], kb[:], AF.Sin,
                                     bias=shift_sb[:])
                nc.scalar.activation(cos_t[:], kb[:], AF.Sin,
                                     scale=sigma_sb[:], bias=tau_sb[:])
                pg0 = pzb.tile([27, 1], F32, tag="pg0")
                pg1 = pzb.tile([27, 1], F32, tag="pg1")
                nc.tensor.matmul(pg0[:], selcos_sb[:], cos_t[:],
                                 start=True, stop=True)
                nc.tensor.matmul(pg1[:], selsin_sb[:], sin_t[:],
                                 start=True, stop=True)
                nc.vector.tensor_copy(bg[:, 0:1], pg0[:])
                nc.vector.tensor_copy(bg[:, 1:2], pg1[:])
                # M_re = cosC*cbG + sinC*sbG ; M_imn = cosC*sbG - sinC*cbG
                nc.vector.tensor_scalar(mt1[:], cosC_sb[:], bg[:, 0:1], None,
                                        op0=ALU.mult)
                nc.vector.tensor_scalar(mt2[:], sinC_sb[:], bg[:, 1:2], None,
                                        op0=ALU.mult)
                nc.vector.tensor_tensor(m_re[:], mt1[:], mt2[:], op=ALU.add)
                nc.vector.tensor_scalar(mt1[:], cosC_sb[:], bg[:, 1:2], None,
                                        op0=ALU.mult)
                nc.vector.tensor_scalar(mt2[:], sinC_sb[:], bg[:, 0:1], None,
                                        op0=ALU.mult)
                nc.vector.tensor_tensor(m_imn[:], mt1[:], mt2[:],
                                        op=ALU.subtract)
                # w2d halves: psum partitions (c', o), free uv
                for h in range(2):
                    pw = pzb.tile([128, 9], F32, tag=f"pw{h}")
                    nc.tensor.matmul(pw[:], wtt_re[:, h * 128:(h + 1) * 128],
                                     m_re[:], start=True, stop=False)
                    nc.tensor.matmul(pw[:], wtt_im[:, h * 128:(h + 1) * 128],
                                     m_imn[:], start=False, stop=True)
                    nc.vector.tensor_copy(w2d_sb[:, h * 9:(h + 1) * 9], pw[:])
                # transpose -> [18=(h,uv), 128=(c',o)]
                pwt = pzb.tile([18, 128], F32, tag="pwt")
                nc.tensor.transpose(pwt[:], w2d_sb[:], ident_sb[:])
                pwt_sb = pp.tile([18, 128], F32, tag="pwt_sb")
                nc.vector.tensor_copy(pwt_sb[:], pwt[:])
                for dx in range(3):
                    nc.vector.memset(lhsT_c[dx][:].bitcast(F32), 0.0)
                for dx in range(3):
                    for dy in range(3):
                        src = pwt_sb[dy * 3 + dx::9, :] \
                            .rearrange("h (cp o) -> h cp o", o=O)
                        for ys in range(SROWS):
                            yw = ys + dy
                            nc.sync.dma_start(
                                lhsT_c[dx][yw::8, ys * O:(ys + 1) * O],
                                src.bitcast(mdt))

            # ---- phase D: the batch-as-groups conv ----
            with (
                tc.tile_pool(name="pd_out", bufs=3) as pso,
                tc.tile_pool(name="pd_psum", bufs=4, space="PSUM") as pcv,
            ):
                for i in range(NSTRIP):
                    y0 = SROWS * i - 1
                    rv = min(SROWS, H - SROWS * i)
                    strip = strip_bufs[i % 3]
                    ys_, ye_ = max(y0, 0), min(y0 + 8, H)   # valid row range
                    if ys_ == y0 and ye_ == y0 + 8:
                        # full strip: dest partitions enumerate (c, yw)
                        # c-major, matching the (c, y, w) source order
                        nc.sync.dma_start(strip[:, 1:1 + W],
                                          x_in.ap()[:, y0:y0 + 8, :]
                                          .bitcast(mdt))
                    else:
                        # edge strip: zero stale rows, then per-row DMAs
                        nc.vector.memset(strip[:].bitcast(F32), 0.0)
                        for y in range(ys_, ye_):
                            nc.sync.dma_start(
                                strip[y - y0::8, 1:1 + W],
                                x_in.ap()[:, y, :].bitcast(mdt))
                    pc = pcv.tile([96, W], F32, tag="pc")
                    for dx in range(3):
                        nc.tensor.matmul(pc[:], lhsT_c[dx][:],
                                         strip[:, dx:dx + W],
                                         start=(dx == 0), stop=(dx == 2))
                    osb = pso.tile([96, W], F32, tag="osb")
                    # PSUM has no DMA route; alternate evac engines
                    if i % 2 == 0:
                        nc.scalar.activation(osb[0:rv * O, :], pc[0:rv * O, :],
                                             AF.Copy)
                    else:
                        nc.vector.tensor_copy(osb[0:rv * O, :], pc[0:rv * O, :])
                    dst = out_t.ap()[:, SROWS * i: SROWS * i + rv, :] \
                        .rearrange("o y w -> y o w")
                    nc.gpsimd.dma_start(dst, osb[0:rv * O, :])

    nc.compile()
    return nc


_NC_CACHE = {}


def _get_nc(key=(True, True)):
    if key not in _NC_CACHE:
        _NC_CACHE[key] = build_nc(*key)
    return _NC_CACHE[key]


def _install_ntff_hook():
    """Shim the missing antenv.axon_hooks so trace=True can profile."""
    try:
        import antenv.axon_hooks  # noqa: F401
        return
    except ImportError:
        pass
    import types

    import antenv

    if "/root/.axon_site" not in sys.path:
        sys.path.insert(0, "/root/.axon_site")
    from trn_agent_boot.trn_boot import _ntff_profile_via_ctypes

    hook = _ntff_profile_via_ctypes("/opt/axon/libaxon_pjrt.so")
    m = types.ModuleType("antenv.axon_hooks")
    holder = {"h": hook}
    m.get_axon_ntff_profile_hook = lambda: holder["h"]
    m.set_axon_ntff_profile_hook = lambda h: holder.__setitem__("h", h)
    sys.modules["antenv.axon_hooks"] = m
    antenv.axon_hooks = m


def run_kernel(inputs, trace=False, trace_kwargs=None):
    nc = _get_nc()
    if trace:
        try:
            _install_ntff_hook()
        except Exception as e:
            print(f"ntff hook install failed ({e}); tracing may be skipped")
    x = np.asarray(inputs["x"], np.float32)
    shared = {
        "w_fft_real": np.ascontiguousarray(inputs["w_fft_real"], np.float32),
        "w_fft_imag": np.ascontiguousarray(inputs["w_fft_imag"], np.float32),
        "rot_w": np.ascontiguousarray(inputs["rot_w"], np.float32),
        "bn_gamma": np.ascontiguousarray(inputs["bn_gamma"], np.float32),
        "bn_beta": np.ascontiguousarray(inputs["bn_beta"], np.float32),
    }
    in_maps = [dict(x=np.ascontiguousarray(x[b]), **shared) for b in range(B)]
    kw = {}
    if trace:
        kw = dict(trace=True, **(trace_kwargs or {}))
    res = run_bass_kernel_spmd(nc, in_maps, list(range(NCORES)), **kw)
    out = np.stack([res.results[b]["out"] for b in range(B)], axis=0)
    return out, res


def kernel(**inputs):
    out, _ = run_kernel(inputs)
    return out


# revision 21
# speedup vs baseline: 2.1602x; 1.1854x over previous
"""Trainium2 Bass kernel for nn_CrossDConv (dense_cnn).

Math (per batch sample b, see reference):
  z = rot_w @ x + rot_b (1x1 conv, 3 out ch), BN over (B,H,W) batch stats,
  angles = spatial mean of z_norm, angle = tanh(sum_i angles)*pi/4,
  s = cos(angle); the 3x3x3 FFT-domain weight tensor is phase-rotated by
  exp(-i*beta*G) with beta = 2*pi*s/3, inverse-FFT'd, mid-slice taken ->
  per-sample 3x3 2D kernels; then a batch-as-groups conv2d (pad 1).

Sharding: data-parallel over B across 8 NeuronCores, one sample per core.
Cross-core work: only the BN batch statistics (an AllReduce of 6 floats).

Device pipeline per core:
  A) stream x (16,512,512) into SBUF in (c, y%8)-partition layout; per
     8-row chunk compute z0 = rot_w@x via a block-diagonal matmul
     (K=128=(c,p), M=24=(i,p)), then Square+accum (ACT) and sum (DVE)
     to get per-sample sum(z0) and sum(z0^2).  rot_b cancels in the BN
     algebra so it is never needed on device.
  B) AllReduce (add) of [S1[i], S2[i]] over the 8 cores.
  C) tiny-op chain: var -> rsqrt -> angles -> tanh -> s=cos(angle);
     build the 27x9 complex iFFT/phase matrix M from sin/cos LUT calls,
     contract with the (27,256) transposed FFT weights (2 small matmuls
     per half) -> w2d (256,9); PE-transpose + 54 small DMAs scatter the
     banded conv lhsT matrices (3 of them, one per kernel column dx).
  D) conv: 86 row-strips of 6 output rows; for each, copy 8 input rows
     from resident x into a (c,yw)-partition strip tile, run 3
     accumulating f32r matmuls (K=128, M=96=(ys,o), N=512), DMA the
     PSUM tile straight to HBM.
"""

import sys

for _p in ("/opt/trn_rl_repo", "/root/.axon_site/_ro/trn_rl_repo"):
    if _p not in sys.path:
        sys.path.insert(0, _p)

import numpy as np

import concourse.bacc as bacc
import concourse.mybir as mybir
import concourse.tile as tile
from concourse.bass_utils import run_bass_kernel_spmd

F32 = mybir.dt.float32
F32R = mybir.dt.float32r
AF = mybir.ActivationFunctionType
ALU = mybir.AluOpType
AX = mybir.AxisListType

B, C, O, K, H, W = 8, 16, 16, 3, 512, 512
NCORES = 8
HWPIX = H * W                    # 262144
NPIX = B * HWPIX                 # 2097152
BN_EPS = 1e-5
NCHUNK = H // 8                  # 64 chunks of 8 rows
WPAD = W + 2                     # row layout: [0pad, x0..x511, 0pad]
SROWS = 6                        # output rows per conv strip
NSTRIP = (H + SROWS - 1) // SROWS  # 86 (last strip has 2 valid rows)
PI = float(np.pi)


def _consts():
    """Host-precomputed, input-independent constants (baked into the NEFF)."""
    g = np.array([0, 1, -1], np.int64)          # 3*fftfreq(3)
    j1, j2, j3 = np.meshgrid(np.arange(3), np.arange(3), np.arange(3),
                             indexing="ij")
    G = (g[j1] + g[j2] + g[j3]).reshape(27)     # in [-3, 3]

    sel_cos = np.zeros((4, 27), np.float32)
    sel_sin = np.zeros((4, 27), np.float32)
    for j in range(27):
        a = abs(G[j])
        sel_cos[a, j] = 1.0
        if G[j] != 0:
            sgn = float(np.sign(G[j]))
            # sin_t[2] holds sin(2b - pi) = -sin(2b): fold the flip in here
            sel_sin[a, j] = -sgn if a == 2 else sgn

    u = np.arange(3)[None, :, None]
    v = np.arange(3)[None, None, :]
    cang = (2.0 * np.pi / 3.0) * (j1.reshape(27, 1, 1) * 1
                                  + j2.reshape(27, 1, 1) * u
                                  + j3.reshape(27, 1, 1) * v)
    cang = cang.reshape(27, 9)
    cosC = (np.cos(cang) / 27.0).astype(np.float32)
    sinC = (np.sin(cang) / 27.0).astype(np.float32)

    kconst = ((2.0 * np.pi / 3.0) * np.arange(4)).reshape(4, 1).astype(np.float32)
    shift_s = np.array([0.0, 0.0, -np.pi, -2.0 * np.pi], np.float32).reshape(4, 1)
    sigma = np.array([1.0, -1.0, -1.0, 1.0], np.float32).reshape(4, 1)
    tau = np.array([np.pi / 2, np.pi / 2, np.pi / 2, -1.5 * np.pi],
                   np.float32).reshape(4, 1)

    foldI = np.zeros((24, 3), np.float32)       # (i,p) -> i
    for k in range(24):
        foldI[k, k // 8] = 1.0

    ident = np.eye(128, dtype=np.float32)
    return dict(sel_cos=sel_cos, sel_sin=sel_sin, cosC=cosC, sinC=sinC,
                kconst=kconst, shift_s=shift_s, sigma=sigma, tau=tau,
                foldI=foldI, ident=ident)


def build_nc(use_f32r_conv=True, use_f32r_z0=True):
    nc = bacc.Bacc("TRN2", target_bir_lowering=False, debug=False,
                   num_devices=NCORES)

    x_in = nc.dram_tensor("x", [C, H, W], F32, kind="ExternalInput")
    wfr_in = nc.dram_tensor("w_fft_real", [O, C, K, K, K], F32,
                            kind="ExternalInput")
    wfi_in = nc.dram_tensor("w_fft_imag", [O, C, K, K, K], F32,
                            kind="ExternalInput")
    rotw_in = nc.dram_tensor("rot_w", [3, C], F32, kind="ExternalInput")
    gam_in = nc.dram_tensor("bn_gamma", [3], F32, kind="ExternalInput")
    bet_in = nc.dram_tensor("bn_beta", [3], F32, kind="ExternalInput")
    out_t = nc.dram_tensor("out", [O, H, W], F32, kind="ExternalOutput")

    cc_in = nc.dram_tensor("cc_in", [1, 8], F32)    # internal bounce
    cc_out = nc.dram_tensor("cc_out", [1, 8], F32)

    cst = _consts()
    c_selcos = nc.inline_tensor(cst["sel_cos"], "c_selcos")
    c_selsin = nc.inline_tensor(cst["sel_sin"], "c_selsin")
    c_cosC = nc.inline_tensor(cst["cosC"], "c_cosC")
    c_sinC = nc.inline_tensor(cst["sinC"], "c_sinC")
    c_kconst = nc.inline_tensor(cst["kconst"], "c_kconst")
    c_shift = nc.inline_tensor(cst["shift_s"], "c_shift")
    c_sigma = nc.inline_tensor(cst["sigma"], "c_sigma")
    c_tau = nc.inline_tensor(cst["tau"], "c_tau")
    c_foldI = nc.inline_tensor(cst["foldI"], "c_foldI")
    c_ident = nc.inline_tensor(cst["ident"], "c_ident")

    mdt = F32R if use_f32r_conv else F32

    with tile.TileContext(nc) as tc:
        with tc.tile_pool(name="persist", bufs=1) as pp:
            lhsT_z = pp.tile([128, 24], mdt)
            wtt_re = pp.tile([27, 256], F32)
            wtt_im = pp.tile([27, 256], F32)
            s1cols = pp.tile([24, NCHUNK], F32)
            s2cols = pp.tile([24, NCHUNK], F32)
            ss = pp.tile([24, 2], F32)
            loc_s = pp.tile([1, 8], F32)
            tot_s = pp.tile([1, 8], F32)
            gam_sb = pp.tile([1, 3], F32)
            bet_sb = pp.tile([1, 3], F32)
            ident_sb = pp.tile([128, 128], F32)
            selcos_sb = pp.tile([4, 27], F32)
            selsin_sb = pp.tile([4, 27], F32)
            cosC_sb = pp.tile([27, 9], F32)
            sinC_sb = pp.tile([27, 9], F32)
            kconst_sb = pp.tile([4, 1], F32)
            shift_sb = pp.tile([4, 1], F32)
            sigma_sb = pp.tile([4, 1], F32)
            tau_sb = pp.tile([4, 1], F32)
            w2d_sb = pp.tile([128, 18], F32)
            lhsT_c = [pp.tile([128, 96], mdt, tag=f"lhsTc{dx}",
                              name=f"lhsT_c{dx}")
                      for dx in range(3)]
            sc3 = pp.tile([1, 3], F32, tag="sc3a")   # phase-C temporaries
            sc3b = pp.tile([1, 3], F32, tag="sc3b")
            sc3c = pp.tile([1, 3], F32, tag="sc3c")
            sc1 = pp.tile([1, 1], F32, tag="sc1a")
            sc1b = pp.tile([1, 1], F32, tag="sc1b")
            kb = pp.tile([4, 1], F32, tag="kb")
            s4 = pp.tile([4, 1], F32, tag="s4")
            sin_t = pp.tile([4, 1], F32, tag="sint")
            cos_t = pp.tile([4, 1], F32, tag="cost")
            bg = pp.tile([27, 2], F32, tag="bg")
            m_re = pp.tile([27, 9], F32, tag="mre")
            m_imn = pp.tile([27, 9], F32, tag="mimn")
            mt1 = pp.tile([27, 9], F32, tag="mt1")
            mt2 = pp.tile([27, 9], F32, tag="mt2")
            eps_sb = pp.tile([1, 1], F32, tag="eps_sb")
            nqpi_sb = pp.tile([1, 1], F32, tag="nqpi_sb")
            hpi_sb = pp.tile([1, 1], F32, tag="hpi_sb")
            nc.vector.memset(eps_sb[:], BN_EPS)
            nc.vector.memset(nqpi_sb[:], -PI / 4.0)
            nc.vector.memset(hpi_sb[:], PI / 2.0)

            # ---- one-time setup ----
            nc.sync.dma_start(ident_sb[:], c_ident.ap())
            nc.sync.dma_start(selcos_sb[:], c_selcos.ap())
            nc.sync.dma_start(selsin_sb[:], c_selsin.ap())
            nc.sync.dma_start(cosC_sb[:], c_cosC.ap())
            nc.sync.dma_start(sinC_sb[:], c_sinC.ap())
            nc.sync.dma_start(kconst_sb[:], c_kconst.ap())
            nc.sync.dma_start(shift_sb[:], c_shift.ap())
            nc.sync.dma_start(sigma_sb[:], c_sigma.ap())
            nc.sync.dma_start(tau_sb[:], c_tau.ap())
            nc.sync.dma_start(gam_sb[:], gam_in.ap().unsqueeze(0))
            nc.sync.dma_start(bet_sb[:], bet_in.ap().unsqueeze(0))
            # transposed FFT weights: [27, (c,o)] so the w2d matmul's PSUM
            # partition layout is (c', o) (c-high folded into the free dim).
            # One DMA per channel keeps every AP within the 3-dim DMA limit.
            wtt_src_re = wfr_in.ap().rearrange("o c a b d -> c (a b d) o")
            wtt_src_im = wfi_in.ap().rearrange("o c a b d -> c (a b d) o")
            for c in range(C):
                nc.sync.dma_start(wtt_re[:, c * O:(c + 1) * O],
                                  wtt_src_re[c])
                nc.sync.dma_start(wtt_im[:, c * O:(c + 1) * O],
                                  wtt_src_im[c])
            # block-diagonal rot_w: lhsT_z[(c,p), (i*8+p)] = rot_w[i, c]
            nc.vector.memset(lhsT_z[:].bitcast(F32), 0.0)
            rot_co = rotw_in.ap().rearrange("i c -> c i")
            for p in range(8):
                nc.sync.dma_start(lhsT_z[p::8, p:24:8], rot_co.bitcast(mdt))
            nc.vector.memset(loc_s[:], 0.0)
            # persistent conv strip buffers (manual 3-way rotation) with
            # zero pad columns written once
            strip_bufs = [pp.tile([128, WPAD], mdt, name=f"stripb{k}")
                          for k in range(12)]
            for sb in strip_bufs:
                nc.vector.memset(sb[:, 0:1].bitcast(F32), 0.0)
                nc.vector.memset(sb[:, WPAD - 1:WPAD].bitcast(F32), 0.0)

            # ---- phase A: load x + z0 statistics ----
            with (
                tc.tile_pool(name="pa_psum", bufs=8, space="PSUM") as pza,
                tc.tile_pool(name="pa_x", bufs=6) as pax,
                tc.tile_pool(name="pa_scr", bufs=4) as psc,
            ):
                for g in range(NCHUNK):
                    xch = pax.tile([128, W], mdt, tag="xchunk")
                    src = x_in.ap()[:, 8 * g: 8 * g + 8, :]
                    nc.sync.dma_start(xch[:], src.bitcast(mdt))
                    z0 = pza.tile([24, W], F32, tag="z0")
                    nc.tensor.matmul(z0[:], lhsT_z[:], xch[:],
                                     start=True, stop=True)
                    scr = psc.tile([24, W], mybir.dt.bfloat16, tag="scr")
                    nc.scalar.activation(scr[:], z0[:], AF.Square,
                                         accum_out=s2cols[:, g:g + 1])
                    nc.vector.reduce_sum(s1cols[:, g:g + 1], z0[:], axis=AX.X)

            # ---- phase A2 + B: fold + AllReduce ----
            with tc.tile_pool(name="pb_psum", bufs=1, space="PSUM") as pzb:
                nc.vector.reduce_sum(ss[:, 0:1], s1cols[:], axis=AX.X)
                nc.vector.reduce_sum(ss[:, 1:2], s2cols[:], axis=AX.X)
                pf = pzb.tile([3, 2], F32, tag="pf")
                foldI_sb = pp.tile([24, 3], F32, tag="foldI")
                nc.sync.dma_start(foldI_sb[:], c_foldI.ap())
                nc.tensor.matmul(pf[:], foldI_sb[:], ss[:],
                                 start=True, stop=True)
                pf_sb = pp.tile([3, 2], F32, tag="pf_sb")
                nc.vector.tensor_copy(pf_sb[:], pf[:])
                # interleaved (S1[0],S2[0],S1[1],S2[1],S1[2],S2[2])
                nc.sync.dma_start(loc_s[:, 0:6], pf_sb[:])
                nc.sync.dma_start(cc_in.ap(), loc_s[:])
                nc.gpsimd.collective_compute(
                    "AllReduce", ALU.add,
                    replica_groups=[list(range(NCORES))],
                    ins=[cc_in.ap()], outs=[cc_out.ap()])
                nc.sync.dma_start(tot_s[:], cc_out.ap())

                # ---- phase C: scalars -> rotation -> w2d -> conv lhsT ----
                t1 = tot_s[:, 0:6:2]     # sum z0   (over batch)
                t2 = tot_s[:, 1:6:2]     # sum z0^2 (over batch)
                nc.vector.tensor_scalar_mul(sc3[:], t1, 1.0 / NPIX)   # m1
                nc.vector.tensor_scalar_mul(sc3b[:], t2, 1.0 / NPIX)  # e2
                nc.vector.tensor_tensor(sc3c[:], sc3[:], sc3[:], op=ALU.mult)
                nc.vector.tensor_tensor(sc3b[:], sc3b[:], sc3c[:],
                                        op=ALU.subtract)              # var
                nc.scalar.activation(sc3b[:], sc3b[:], AF.Sqrt,
                                     bias=eps_sb[:])
                nc.vector.reciprocal(sc3b[:], sc3b[:])                # rsqrt
                nc.vector.tensor_tensor(sc3b[:], sc3b[:], gam_sb[:],
                                        op=ALU.mult)                  # inv
                nc.vector.tensor_scalar_mul(sc3c[:], loc_s[:, 0:6:2],
                                            1.0 / HWPIX)              # s1h
                nc.vector.tensor_tensor(sc3c[:], sc3c[:], sc3[:],
                                        op=ALU.subtract)              # diff
                nc.vector.tensor_tensor(sc3c[:], sc3c[:], sc3b[:],
                                        op=ALU.mult)
                nc.vector.tensor_tensor(sc3c[:], sc3c[:], bet_sb[:],
                                        op=ALU.add)                   # angles
                nc.vector.reduce_sum(sc1[:], sc3c[:], axis=AX.X)      # a
                nc.scalar.activation(sc1b[:], sc1[:], AF.Tanh)
                # s = cos(tanh(a)*pi/4) = sin(pi/2 - (pi/4)*tanh(a))
                nc.scalar.activation(sc1[:], sc1b[:], AF.Sin,
                                     scale=nqpi_sb[:], bias=hpi_sb[:])
                nc.gpsimd.partition_broadcast(s4[:], sc1[:])
                nc.vector.tensor_tensor(kb[:], kconst_sb[:], s4[:],
                                        op=ALU.mult)                  # k*beta
                nc.scalar.activation(sin_t[:], kb[:], AF.Sin,
                                     bias=shift_sb[:])
                nc.scalar.activation(cos_t[:], kb[:], AF.Sin,
                                     scale=sigma_sb[:], bias=tau_sb[:])
                pg0 = pzb.tile([27, 1], F32, tag="pg0")
                pg1 = pzb.tile([27, 1], F32, tag="pg1")
                nc.tensor.matmul(pg0[:], selcos_sb[:], cos_t[:],
                                 start=True, stop=True)
                nc.tensor.matmul(pg1[:], selsin_sb[:], sin_t[:],
                                 start=True, stop=True)
                nc.vector.tensor_copy(bg[:, 0:1], pg0[:])
                nc.vector.tensor_copy(bg[:, 1:2], pg1[:])
                # M_re = cosC*cbG + sinC*sbG ; M_imn = cosC*sbG - sinC*cbG
                nc.vector.tensor_scalar(mt1[:], cosC_sb[:], bg[:, 0:1], None,
                                        op0=ALU.mult)
                nc.vector.tensor_scalar(mt2[:], sinC_sb[:], bg[:, 1:2], None,
                                        op0=ALU.mult)
                nc.vector.tensor_tensor(m_re[:], mt1[:], mt2[:], op=ALU.add)
                nc.vector.tensor_scalar(mt1[:], cosC_sb[:], bg[:, 1:2], None,
                                        op0=ALU.mult)
                nc.vector.tensor_scalar(mt2[:], sinC_sb[:], bg[:, 0:1], None,
                                        op0=ALU.mult)
                nc.vector.tensor_tensor(m_imn[:], mt1[:], mt2[:],
                                        op=ALU.subtract)
                # w2d halves: psum partitions (c', o), free uv
                for h in range(2):
                    pw = pzb.tile([128, 9], F32, tag=f"pw{h}")
                    nc.tensor.matmul(pw[:], wtt_re[:, h * 128:(h + 1) * 128],
                                     m_re[:], start=True, stop=False)
                    nc.tensor.matmul(pw[:], wtt_im[:, h * 128:(h + 1) * 128],
                                     m_imn[:], start=False, stop=True)
                    nc.vector.tensor_copy(w2d_sb[:, h * 9:(h + 1) * 9], pw[:])
                # transpose -> [18=(h,uv), 128=(c',o)]
                pwt = pzb.tile([18, 128], F32, tag="pwt")
                nc.tensor.transpose(pwt[:], w2d_sb[:], ident_sb[:])
                pwt_sb = pp.tile([18, 128], F32, tag="pwt_sb")
                nc.vector.tensor_copy(pwt_sb[:], pwt[:])
                for dx in range(3):
                    nc.vector.memset(lhsT_c[dx][:].bitcast(F32), 0.0)
                for dx in range(3):
                    for dy in range(3):
                        src = pwt_sb[dy * 3 + dx::9, :] \
                            .rearrange("h (cp o) -> h cp o", o=O)
                        for ys in range(SROWS):
                            yw = ys + dy
                            nc.sync.dma_start(
                                lhsT_c[dx][yw::8, ys * O:(ys + 1) * O],
                                src.bitcast(mdt))

            # ---- phase D: the batch-as-groups conv ----
            with (
                tc.tile_pool(name="pd_out", bufs=4) as pso,
                tc.tile_pool(name="pd_psum", bufs=8, space="PSUM") as pcv,
            ):
                for i in range(NSTRIP):
                    y0 = SROWS * i - 1
                    rv = min(SROWS, H - SROWS * i)
                    strip = strip_bufs[i % 12]
                    ys_, ye_ = max(y0, 0), min(y0 + 8, H)   # valid row range
                    if ys_ == y0 and ye_ == y0 + 8:
                        # full strip: dest partitions enumerate (c, yw)
                        # c-major, matching the (c, y, w) source order
                        nc.sync.dma_start(strip[:, 1:1 + W],
                                          x_in.ap()[:, y0:y0 + 8, :]
                                          .bitcast(mdt))
                    else:
                        # edge strip: zero stale rows, then per-row DMAs
                        nc.vector.memset(strip[:].bitcast(F32), 0.0)
                        for y in range(ys_, ye_):
                            nc.sync.dma_start(
                                strip[y - y0::8, 1:1 + W],
                                x_in.ap()[:, y, :].bitcast(mdt))
                    pc = pcv.tile([96, W], F32, tag="pc")
                    for dx in range(3):
                        nc.tensor.matmul(pc[:], lhsT_c[dx][:],
                                         strip[:, dx:dx + W],
                                         start=(dx == 0), stop=(dx == 2))
                    osb = pso.tile([96, W], F32, tag="osb")
                    # PSUM has no DMA route; alternate evac engines
                    if i % 2 == 0:
                        nc.scalar.activation(osb[0:rv * O, :], pc[0:rv * O, :],
                                             AF.Copy)
                    else:
                        nc.vector.tensor_copy(osb[0:rv * O, :], pc[0:rv * O, :])
                    dst = out_t.ap()[:, SROWS * i: SROWS * i + rv, :] \
                        .rearrange("o y w -> y o w")
                    nc.gpsimd.dma_start(dst, osb[0:rv * O, :])

    nc.compile()
    return nc


_NC_CACHE = {}


def _get_nc(key=(True, True)):
    if key not in _NC_CACHE:
        _NC_CACHE[key] = build_nc(*key)
    return _NC_CACHE[key]


def _install_ntff_hook():
    """Shim the missing antenv.axon_hooks so trace=True can profile."""
    try:
        import antenv.axon_hooks  # noqa: F401
        return
    except ImportError:
        pass
    import types

    import antenv

    if "/root/.axon_site" not in sys.path:
        sys.path.insert(0, "/root/.axon_site")
    from trn_agent_boot.trn_boot import _ntff_profile_via_ctypes

    hook = _ntff_profile_via_ctypes("/opt/axon/libaxon_pjrt.so")
    m = types.ModuleType("antenv.axon_hooks")
    holder = {"h": hook}
    m.get_axon_ntff_profile_hook = lambda: holder["h"]
    m.set_axon_ntff_profile_hook = lambda h: holder.__setitem__("h", h)
    sys.modules["antenv.axon_hooks"] = m
    antenv.axon_hooks = m


def run_kernel(inputs, trace=False, trace_kwargs=None):
    nc = _get_nc()
    if trace:
        try:
            _install_ntff_hook()
        except Exception as e:
            print(f"ntff hook install failed ({e}); tracing may be skipped")
    x = np.asarray(inputs["x"], np.float32)
    shared = {
        "w_fft_real": np.ascontiguousarray(inputs["w_fft_real"], np.float32),
        "w_fft_imag": np.ascontiguousarray(inputs["w_fft_imag"], np.float32),
        "rot_w": np.ascontiguousarray(inputs["rot_w"], np.float32),
        "bn_gamma": np.ascontiguousarray(inputs["bn_gamma"], np.float32),
        "bn_beta": np.ascontiguousarray(inputs["bn_beta"], np.float32),
    }
    in_maps = [dict(x=np.ascontiguousarray(x[b]), **shared) for b in range(B)]
    kw = {}
    if trace:
        kw = dict(trace=True, **(trace_kwargs or {}))
    res = run_bass_kernel_spmd(nc, in_maps, list(range(NCORES)), **kw)
    out = np.stack([res.results[b]["out"] for b in range(B)], axis=0)
    return out, res


def kernel(**inputs):
    out, _ = run_kernel(inputs)
    return out


# revision 22
# speedup vs baseline: 2.2364x; 1.0353x over previous
"""Trainium2 Bass kernel for nn_CrossDConv (dense_cnn).

Math (per batch sample b, see reference):
  z = rot_w @ x + rot_b (1x1 conv, 3 out ch), BN over (B,H,W) batch stats,
  angles = spatial mean of z_norm, angle = tanh(sum_i angles)*pi/4,
  s = cos(angle); the 3x3x3 FFT-domain weight tensor is phase-rotated by
  exp(-i*beta*G) with beta = 2*pi*s/3, inverse-FFT'd, mid-slice taken ->
  per-sample 3x3 2D kernels; then a batch-as-groups conv2d (pad 1).

Sharding: data-parallel over B across 8 NeuronCores, one sample per core.
Cross-core work: only the BN batch statistics (an AllReduce of 6 floats).

Device pipeline per core:
  A) stream x (16,512,512) into SBUF in (c, y%8)-partition layout; per
     8-row chunk compute z0 = rot_w@x via a block-diagonal matmul
     (K=128=(c,p), M=24=(i,p)), then Square+accum (ACT) and sum (DVE)
     to get per-sample sum(z0) and sum(z0^2).  rot_b cancels in the BN
     algebra so it is never needed on device.
  B) AllReduce (add) of [S1[i], S2[i]] over the 8 cores.
  C) tiny-op chain: var -> rsqrt -> angles -> tanh -> s=cos(angle);
     build the 27x9 complex iFFT/phase matrix M from sin/cos LUT calls,
     contract with the (27,256) transposed FFT weights (2 small matmuls
     per half) -> w2d (256,9); PE-transpose + 54 small DMAs scatter the
     banded conv lhsT matrices (3 of them, one per kernel column dx).
  D) conv: 86 row-strips of 6 output rows; for each, copy 8 input rows
     from resident x into a (c,yw)-partition strip tile, run 3
     accumulating f32r matmuls (K=128, M=96=(ys,o), N=512), DMA the
     PSUM tile straight to HBM.
"""

import sys

for _p in ("/opt/trn_rl_repo", "/root/.axon_site/_ro/trn_rl_repo"):
    if _p not in sys.path:
        sys.path.insert(0, _p)

import numpy as np

import concourse.bacc as bacc
import concourse.mybir as mybir
import concourse.tile as tile
from concourse.bass_utils import run_bass_kernel_spmd

F32 = mybir.dt.float32
F32R = mybir.dt.float32r
AF = mybir.ActivationFunctionType
ALU = mybir.AluOpType
AX = mybir.AxisListType

B, C, O, K, H, W = 8, 16, 16, 3, 512, 512
NCORES = 8
HWPIX = H * W                    # 262144
NPIX = B * HWPIX                 # 2097152
BN_EPS = 1e-5
NCHUNK = H // 8                  # 64 chunks of 8 rows
WPAD = W + 2                     # row layout: [0pad, x0..x511, 0pad]
SROWS = 6                        # output rows per conv strip
NSTRIP = (H + SROWS - 1) // SROWS  # 86 (last strip has 2 valid rows)
PI = float(np.pi)


def _consts():
    """Host-precomputed, input-independent constants (baked into the NEFF)."""
    g = np.array([0, 1, -1], np.int64)          # 3*fftfreq(3)
    j1, j2, j3 = np.meshgrid(np.arange(3), np.arange(3), np.arange(3),
                             indexing="ij")
    G = (g[j1] + g[j2] + g[j3]).reshape(27)     # in [-3, 3]

    sel_cos = np.zeros((4, 27), np.float32)
    sel_sin = np.zeros((4, 27), np.float32)
    for j in range(27):
        a = abs(G[j])
        sel_cos[a, j] = 1.0
        if G[j] != 0:
            sgn = float(np.sign(G[j]))
            # sin_t[2] holds sin(2b - pi) = -sin(2b): fold the flip in here
            sel_sin[a, j] = -sgn if a == 2 else sgn

    u = np.arange(3)[None, :, None]
    v = np.arange(3)[None, None, :]
    cang = (2.0 * np.pi / 3.0) * (j1.reshape(27, 1, 1) * 1
                                  + j2.reshape(27, 1, 1) * u
                                  + j3.reshape(27, 1, 1) * v)
    cang = cang.reshape(27, 9)
    cosC = (np.cos(cang) / 27.0).astype(np.float32)
    sinC = (np.sin(cang) / 27.0).astype(np.float32)

    kconst = ((2.0 * np.pi / 3.0) * np.arange(4)).reshape(4, 1).astype(np.float32)
    shift_s = np.array([0.0, 0.0, -np.pi, -2.0 * np.pi], np.float32).reshape(4, 1)
    sigma = np.array([1.0, -1.0, -1.0, 1.0], np.float32).reshape(4, 1)
    tau = np.array([np.pi / 2, np.pi / 2, np.pi / 2, -1.5 * np.pi],
                   np.float32).reshape(4, 1)

    foldI = np.zeros((24, 3), np.float32)       # (i,p) -> i
    for k in range(24):
        foldI[k, k // 8] = 1.0

    ident = np.eye(128, dtype=np.float32)
    return dict(sel_cos=sel_cos, sel_sin=sel_sin, cosC=cosC, sinC=sinC,
                kconst=kconst, shift_s=shift_s, sigma=sigma, tau=tau,
                foldI=foldI, ident=ident)


def build_nc(use_f32r_conv=True, use_f32r_z0=True):
    nc = bacc.Bacc("TRN2", target_bir_lowering=False, debug=False,
                   num_devices=NCORES)

    x_in = nc.dram_tensor("x", [C, H, W], F32, kind="ExternalInput")
    wfr_in = nc.dram_tensor("w_fft_real", [O, C, K, K, K], F32,
                            kind="ExternalInput")
    wfi_in = nc.dram_tensor("w_fft_imag", [O, C, K, K, K], F32,
                            kind="ExternalInput")
    rotw_in = nc.dram_tensor("rot_w", [3, C], F32, kind="ExternalInput")
    gam_in = nc.dram_tensor("bn_gamma", [3], F32, kind="ExternalInput")
    bet_in = nc.dram_tensor("bn_beta", [3], F32, kind="ExternalInput")
    out_t = nc.dram_tensor("out", [O, H, W], F32, kind="ExternalOutput")

    cc_in = nc.dram_tensor("cc_in", [1, 8], F32)    # internal bounce
    cc_out = nc.dram_tensor("cc_out", [1, 8], F32)

    cst = _consts()
    c_selcos = nc.inline_tensor(cst["sel_cos"], "c_selcos")
    c_selsin = nc.inline_tensor(cst["sel_sin"], "c_selsin")
    c_cosC = nc.inline_tensor(cst["cosC"], "c_cosC")
    c_sinC = nc.inline_tensor(cst["sinC"], "c_sinC")
    c_kconst = nc.inline_tensor(cst["kconst"], "c_kconst")
    c_shift = nc.inline_tensor(cst["shift_s"], "c_shift")
    c_sigma = nc.inline_tensor(cst["sigma"], "c_sigma")
    c_tau = nc.inline_tensor(cst["tau"], "c_tau")
    c_foldI = nc.inline_tensor(cst["foldI"], "c_foldI")
    c_ident = nc.inline_tensor(cst["ident"], "c_ident")

    mdt = F32R if use_f32r_conv else F32

    with tile.TileContext(nc) as tc:
        with tc.tile_pool(name="persist", bufs=1) as pp:
            lhsT_z = pp.tile([128, 24], mdt)
            wtt_re = pp.tile([27, 256], F32)
            wtt_im = pp.tile([27, 256], F32)
            s1cols = pp.tile([24, NCHUNK], F32)
            s2cols = pp.tile([24, NCHUNK], F32)
            ss = pp.tile([24, 2], F32)
            loc_s = pp.tile([1, 8], F32)
            tot_s = pp.tile([1, 8], F32)
            gam_sb = pp.tile([1, 3], F32)
            bet_sb = pp.tile([1, 3], F32)
            ident_sb = pp.tile([128, 128], F32)
            selcos_sb = pp.tile([4, 27], F32)
            selsin_sb = pp.tile([4, 27], F32)
            cosC_sb = pp.tile([27, 9], F32)
            sinC_sb = pp.tile([27, 9], F32)
            kconst_sb = pp.tile([4, 1], F32)
            shift_sb = pp.tile([4, 1], F32)
            sigma_sb = pp.tile([4, 1], F32)
            tau_sb = pp.tile([4, 1], F32)
            w2d_sb = pp.tile([128, 18], F32)
            lhsT_c = [pp.tile([128, 96], mdt, tag=f"lhsTc{dx}",
                              name=f"lhsT_c{dx}")
                      for dx in range(3)]
            sc3 = pp.tile([1, 3], F32, tag="sc3a")   # phase-C temporaries
            sc3b = pp.tile([1, 3], F32, tag="sc3b")
            sc3c = pp.tile([1, 3], F32, tag="sc3c")
            sc1 = pp.tile([1, 1], F32, tag="sc1a")
            sc1b = pp.tile([1, 1], F32, tag="sc1b")
            kb = pp.tile([4, 1], F32, tag="kb")
            s4 = pp.tile([4, 1], F32, tag="s4")
            sin_t = pp.tile([4, 1], F32, tag="sint")
            cos_t = pp.tile([4, 1], F32, tag="cost")
            bg = pp.tile([27, 2], F32, tag="bg")
            m_re = pp.tile([27, 9], F32, tag="mre")
            m_imn = pp.tile([27, 9], F32, tag="mimn")
            mt1 = pp.tile([27, 9], F32, tag="mt1")
            mt2 = pp.tile([27, 9], F32, tag="mt2")
            eps_sb = pp.tile([1, 1], F32, tag="eps_sb")
            nqpi_sb = pp.tile([1, 1], F32, tag="nqpi_sb")
            hpi_sb = pp.tile([1, 1], F32, tag="hpi_sb")
            nc.vector.memset(eps_sb[:], BN_EPS)
            nc.vector.memset(nqpi_sb[:], -PI / 4.0)
            nc.vector.memset(hpi_sb[:], PI / 2.0)

            # ---- one-time setup ----
            nc.sync.dma_start(ident_sb[:], c_ident.ap())
            nc.sync.dma_start(selcos_sb[:], c_selcos.ap())
            nc.sync.dma_start(selsin_sb[:], c_selsin.ap())
            nc.sync.dma_start(cosC_sb[:], c_cosC.ap())
            nc.sync.dma_start(sinC_sb[:], c_sinC.ap())
            nc.sync.dma_start(kconst_sb[:], c_kconst.ap())
            nc.sync.dma_start(shift_sb[:], c_shift.ap())
            nc.sync.dma_start(sigma_sb[:], c_sigma.ap())
            nc.sync.dma_start(tau_sb[:], c_tau.ap())
            nc.sync.dma_start(gam_sb[:], gam_in.ap().unsqueeze(0))
            nc.sync.dma_start(bet_sb[:], bet_in.ap().unsqueeze(0))
            # transposed FFT weights: [27, (c,o)] so the w2d matmul's PSUM
            # partition layout is (c', o) (c-high folded into the free dim).
            # One DMA per channel keeps every AP within the 3-dim DMA limit.
            wtt_src_re = wfr_in.ap().rearrange("o c a b d -> c (a b d) o")
            wtt_src_im = wfi_in.ap().rearrange("o c a b d -> c (a b d) o")
            for c in range(C):
                nc.sync.dma_start(wtt_re[:, c * O:(c + 1) * O],
                                  wtt_src_re[c])
                nc.sync.dma_start(wtt_im[:, c * O:(c + 1) * O],
                                  wtt_src_im[c])
            # block-diagonal rot_w: lhsT_z[(c,p), (i*8+p)] = rot_w[i, c]
            nc.vector.memset(lhsT_z[:].bitcast(F32), 0.0)
            rot_co = rotw_in.ap().rearrange("i c -> c i")
            for p in range(8):
                nc.sync.dma_start(lhsT_z[p::8, p:24:8], rot_co.bitcast(mdt))
            nc.vector.memset(loc_s[:], 0.0)
            # persistent conv strip buffers (manual 3-way rotation) with
            # zero pad columns written once
            strip_bufs = [pp.tile([128, WPAD], mdt, name=f"stripb{k}")
                          for k in range(12)]
            for sb in strip_bufs:
                nc.vector.memset(sb[:, 0:1].bitcast(F32), 0.0)
                nc.vector.memset(sb[:, WPAD - 1:WPAD].bitcast(F32), 0.0)

            # ---- phase A: load x + z0 statistics ----
            QC = 4                         # chunks per stats group
            with (
                tc.tile_pool(name="pa_psum", bufs=2, space="PSUM") as pza,
                tc.tile_pool(name="pa_x", bufs=8) as pax,
                tc.tile_pool(name="pa_scr", bufs=2) as psc,
            ):
                for q in range(NCHUNK // QC):
                    z0 = pza.tile([24, QC * W], F32, tag="z0")
                    for j in range(QC):
                        g = q * QC + j
                        xch = pax.tile([128, W], mdt, tag="xchunk")
                        src = x_in.ap()[:, 8 * g: 8 * g + 8, :]
                        nc.sync.dma_start(xch[:], src.bitcast(mdt))
                        nc.tensor.matmul(z0[:, j * W:(j + 1) * W], lhsT_z[:],
                                         xch[:], start=True, stop=True)
                    scr = psc.tile([24, QC * W], mybir.dt.bfloat16, tag="scr")
                    nc.scalar.activation(scr[:], z0[:], AF.Square,
                                         accum_out=s2cols[:, q:q + 1])
                    nc.vector.reduce_sum(s1cols[:, q:q + 1], z0[:], axis=AX.X)

            # ---- phase A2 + B: fold + AllReduce ----
            with tc.tile_pool(name="pb_psum", bufs=1, space="PSUM") as pzb:
                nc.vector.reduce_sum(ss[:, 0:1], s1cols[:], axis=AX.X)
                nc.vector.reduce_sum(ss[:, 1:2], s2cols[:], axis=AX.X)
                pf = pzb.tile([3, 2], F32, tag="pf")
                foldI_sb = pp.tile([24, 3], F32, tag="foldI")
                nc.sync.dma_start(foldI_sb[:], c_foldI.ap())
                nc.tensor.matmul(pf[:], foldI_sb[:], ss[:],
                                 start=True, stop=True)
                pf_sb = pp.tile([3, 2], F32, tag="pf_sb")
                nc.vector.tensor_copy(pf_sb[:], pf[:])
                # interleaved (S1[0],S2[0],S1[1],S2[1],S1[2],S2[2])
                nc.sync.dma_start(loc_s[:, 0:6], pf_sb[:])
                nc.sync.dma_start(cc_in.ap(), loc_s[:])
                nc.gpsimd.collective_compute(
                    "AllReduce", ALU.add,
                    replica_groups=[list(range(NCORES))],
                    ins=[cc_in.ap()], outs=[cc_out.ap()])
                nc.sync.dma_start(tot_s[:], cc_out.ap())

                # ---- phase C: scalars -> rotation -> w2d -> conv lhsT ----
                t1 = tot_s[:, 0:6:2]     # sum z0   (over batch)
                t2 = tot_s[:, 1:6:2]     # sum z0^2 (over batch)
                nc.vector.tensor_scalar_mul(sc3[:], t1, 1.0 / NPIX)   # m1
                nc.vector.tensor_scalar_mul(sc3b[:], t2, 1.0 / NPIX)  # e2
                nc.vector.tensor_tensor(sc3c[:], sc3[:], sc3[:], op=ALU.mult)
                nc.vector.tensor_tensor(sc3b[:], sc3b[:], sc3c[:],
                                        op=ALU.subtract)              # var
                nc.scalar.activation(sc3b[:], sc3b[:], AF.Sqrt,
                                     bias=eps_sb[:])
                nc.vector.reciprocal(sc3b[:], sc3b[:])                # rsqrt
                nc.vector.tensor_tensor(sc3b[:], sc3b[:], gam_sb[:],
                                        op=ALU.mult)                  # inv
                nc.vector.tensor_scalar_mul(sc3c[:], loc_s[:, 0:6:2],
                                            1.0 / HWPIX)              # s1h
                nc.vector.tensor_tensor(sc3c[:], sc3c[:], sc3[:],
                                        op=ALU.subtract)              # diff
                nc.vector.tensor_tensor(sc3c[:], sc3c[:], sc3b[:],
                                        op=ALU.mult)
                nc.vector.tensor_tensor(sc3c[:], sc3c[:], bet_sb[:],
                                        op=ALU.add)                   # angles
                nc.vector.reduce_sum(sc1[:], sc3c[:], axis=AX.X)      # a
                nc.scalar.activation(sc1b[:], sc1[:], AF.Tanh)
                # s = cos(tanh(a)*pi/4) = sin(pi/2 - (pi/4)*tanh(a))
                nc.scalar.activation(sc1[:], sc1b[:], AF.Sin,
                                     scale=nqpi_sb[:], bias=hpi_sb[:])
                nc.gpsimd.partition_broadcast(s4[:], sc1[:])
                nc.vector.tensor_tensor(kb[:], kconst_sb[:], s4[:],
                                        op=ALU.mult)                  # k*beta
                nc.scalar.activation(sin_t[:], kb[:], AF.Sin,
                                     bias=shift_sb[:])
                nc.scalar.activation(cos_t[:], kb[:], AF.Sin,
                                     scale=sigma_sb[:], bias=tau_sb[:])
                pg0 = pzb.tile([27, 1], F32, tag="pg0")
                pg1 = pzb.tile([27, 1], F32, tag="pg1")
                nc.tensor.matmul(pg0[:], selcos_sb[:], cos_t[:],
                                 start=True, stop=True)
                nc.tensor.matmul(pg1[:], selsin_sb[:], sin_t[:],
                                 start=True, stop=True)
                nc.vector.tensor_copy(bg[:, 0:1], pg0[:])
                nc.vector.tensor_copy(bg[:, 1:2], pg1[:])
                # M_re = cosC*cbG + sinC*sbG ; M_imn = cosC*sbG - sinC*cbG
                nc.vector.tensor_scalar(mt1[:], cosC_sb[:], bg[:, 0:1], None,
                                        op0=ALU.mult)
                nc.vector.tensor_scalar(mt2[:], sinC_sb[:], bg[:, 1:2], None,
                                        op0=ALU.mult)
                nc.vector.tensor_tensor(m_re[:], mt1[:], mt2[:], op=ALU.add)
                nc.vector.tensor_scalar(mt1[:], cosC_sb[:], bg[:, 1:2], None,
                                        op0=ALU.mult)
                nc.vector.tensor_scalar(mt2[:], sinC_sb[:], bg[:, 0:1], None,
                                        op0=ALU.mult)
                nc.vector.tensor_tensor(m_imn[:], mt1[:], mt2[:],
                                        op=ALU.subtract)
                # w2d halves: psum partitions (c', o), free uv
                for h in range(2):
                    pw = pzb.tile([128, 9], F32, tag=f"pw{h}")
                    nc.tensor.matmul(pw[:], wtt_re[:, h * 128:(h + 1) * 128],
                                     m_re[:], start=True, stop=False)
                    nc.tensor.matmul(pw[:], wtt_im[:, h * 128:(h + 1) * 128],
                                     m_imn[:], start=False, stop=True)
                    nc.vector.tensor_copy(w2d_sb[:, h * 9:(h + 1) * 9], pw[:])
                # transpose -> [18=(h,uv), 128=(c',o)]
                pwt = pzb.tile([18, 128], F32, tag="pwt")
                nc.tensor.transpose(pwt[:], w2d_sb[:], ident_sb[:])
                pwt_sb = pp.tile([18, 128], F32, tag="pwt_sb")
                nc.vector.tensor_copy(pwt_sb[:], pwt[:])
                for dx in range(3):
                    nc.vector.memset(lhsT_c[dx][:].bitcast(F32), 0.0)
                _n = 0
                for dx in range(3):
                    for dy in range(3):
                        src = pwt_sb[dy * 3 + dx::9, :] \
                            .rearrange("h (cp o) -> h cp o", o=O)
                        for ys in range(SROWS):
                            yw = ys + dy
                            eng = nc.sync if _n % 2 == 0 else nc.scalar
                            eng.dma_start(
                                lhsT_c[dx][yw::8, ys * O:(ys + 1) * O],
                                src.bitcast(mdt))
                            _n += 1

            # ---- phase D: the batch-as-groups conv ----
            with (
                tc.tile_pool(name="pd_out", bufs=4) as pso,
                tc.tile_pool(name="pd_psum", bufs=8, space="PSUM") as pcv,
            ):
                for i in range(NSTRIP):
                    y0 = SROWS * i - 1
                    rv = min(SROWS, H - SROWS * i)
                    strip = strip_bufs[i % 12]
                    ys_, ye_ = max(y0, 0), min(y0 + 8, H)   # valid row range
                    if ys_ == y0 and ye_ == y0 + 8:
                        # full strip: dest partitions enumerate (c, yw)
                        # c-major, matching the (c, y, w) source order
                        nc.sync.dma_start(strip[:, 1:1 + W],
                                          x_in.ap()[:, y0:y0 + 8, :]
                                          .bitcast(mdt))
                    else:
                        # edge strip: zero stale rows, then per-row DMAs
                        nc.vector.memset(strip[:].bitcast(F32), 0.0)
                        for y in range(ys_, ye_):
                            nc.sync.dma_start(
                                strip[y - y0::8, 1:1 + W],
                                x_in.ap()[:, y, :].bitcast(mdt))
                    pc = pcv.tile([96, W], F32, tag="pc")
                    for dx in range(3):
                        nc.tensor.matmul(pc[:], lhsT_c[dx][:],
                                         strip[:, dx:dx + W],
                                         start=(dx == 0), stop=(dx == 2))
                    osb = pso.tile([96, W], F32, tag="osb")
                    # PSUM has no DMA route; alternate evac engines
                    if i % 2 == 0:
                        nc.scalar.activation(osb[0:rv * O, :], pc[0:rv * O, :],
                                             AF.Copy)
                    else:
                        nc.vector.tensor_copy(osb[0:rv * O, :], pc[0:rv * O, :])
                    dst = out_t.ap()[:, SROWS * i: SROWS * i + rv, :] \
                        .rearrange("o y w -> y o w")
                    nc.gpsimd.dma_start(dst, osb[0:rv * O, :])

    nc.compile()
    return nc


_NC_CACHE = {}


def _get_nc(key=(True, True)):
    if key not in _NC_CACHE:
        _NC_CACHE[key] = build_nc(*key)
    return _NC_CACHE[key]


def _install_ntff_hook():
    """Shim the missing antenv.axon_hooks so trace=True can profile."""
    try:
        import antenv.axon_hooks  # noqa: F401
        return
    except ImportError:
        pass
    import types

    import antenv

    if "/root/.axon_site" not in sys.path:
        sys.path.insert(0, "/root/.axon_site")
    from trn_agent_boot.trn_boot import _ntff_profile_via_ctypes

    hook = _ntff_profile_via_ctypes("/opt/axon/libaxon_pjrt.so")
    m = types.ModuleType("antenv.axon_hooks")
    holder = {"h": hook}
    m.get_axon_ntff_profile_hook = lambda: holder["h"]
    m.set_axon_ntff_profile_hook = lambda h: holder.__setitem__("h", h)
    sys.modules["antenv.axon_hooks"] = m
    antenv.axon_hooks = m


def run_kernel(inputs, trace=False, trace_kwargs=None):
    nc = _get_nc()
    if trace:
        try:
            _install_ntff_hook()
        except Exception as e:
            print(f"ntff hook install failed ({e}); tracing may be skipped")
    x = np.asarray(inputs["x"], np.float32)
    shared = {
        "w_fft_real": np.ascontiguousarray(inputs["w_fft_real"], np.float32),
        "w_fft_imag": np.ascontiguousarray(inputs["w_fft_imag"], np.float32),
        "rot_w": np.ascontiguousarray(inputs["rot_w"], np.float32),
        "bn_gamma": np.ascontiguousarray(inputs["bn_gamma"], np.float32),
        "bn_beta": np.ascontiguousarray(inputs["bn_beta"], np.float32),
    }
    in_maps = [dict(x=np.ascontiguousarray(x[b]), **shared) for b in range(B)]
    kw = {}
    if trace:
        kw = dict(trace=True, **(trace_kwargs or {}))
    res = run_bass_kernel_spmd(nc, in_maps, list(range(NCORES)), **kw)
    out = np.stack([res.results[b]["out"] for b in range(B)], axis=0)
    return out, res


def kernel(**inputs):
    out, _ = run_kernel(inputs)
    return out
